# revision 42
# baseline (speedup 1.0000x reference)
"""ArcMarginProduct + cross-entropy loss, vocab-parallel over 8 NeuronCores.

Math: the reference computes
    cos[b,v] = <x_b/|x_b|, w_v/|w_v|>,  clip to [-1+eps, 1-eps]
    logits   = cos(arccos(cos) + M*onehot(labels))
    loss     = mean(logsumexp(logits, axis=1) - logits[b, label_b])
For v != label_b, cos(arccos(c)) == c, so the only place arccos/cos matter is
the single label column per row -- handled exactly on the host (O(B*D) work).
The device computes, per vocabulary shard, S_partial[b] = sum_v exp(cos[b,v])
(raw, no margin). |cos|<=1 always, so no max-shift is needed for stability.
Host then corrects the label term: S_adj = S - exp(c_label) + exp(c_adj),
loss = mean(log(S_adj) - c_adj).

Sharding: weight columns split V=100000 -> 8 x 12500, padded with zero
columns to 12544 per core (pad contributions are constant and subtracted
exactly on the host).

Device kernel (per core): both operands are L2-normalized ON THE HOST and
shipped as fp8, so the PSUM matmul result is exactly SX*SW*cos and the exp
scale is one scalar constant. Layout: batch rows on PSUM partitions (8
b-tiles of 128), classes on the free axis, so the per-row sum over classes
is a free-axis reduction the ScalarE activation produces for free via
accum_out. Per [128, 2048] class group (4 PSUM banks, double-buffered):
DoubleRow fp8 matmuls (256-deep contraction, x stationary) accumulate over
D; then the group is consumed column-split by TWO engines in parallel --
ScalarE runs Exp+accum_out on the first ACT_COLS columns while VectorE runs
a Schraudolph bit-trick exp (bits(bf16(e^z)) ~= round(A*P + B), ~1.3% rms,
~0.1% mean error) on the rest, with one batched free-axis reduce per b-tile.
No SBUF-side accumulation pass exists at all; the host sums 7 columns per
b-tile. PE warm-up matmuls during the weight-DMA lead-in keep the HAM clock
gate at 8/8 when the real matmuls start.
"""

import math
import sys

if "/opt/trn_rl_repo" not in sys.path:
    sys.path.insert(0, "/opt/trn_rl_repo")

import numpy as np
import ml_dtypes

import concourse.bass as bass
import concourse.mybir as mybir
import concourse.tile as tile
from concourse.bass_utils import run_bass_kernel_spmd

B, D, V = 1024, 512, 100000
NCORES = 8
VS = V // NCORES           # 12500 true classes per core
VSP = 12544                # padded classes per core
KB = D // 128              # 4 contraction blocks
NBT = B // 128             # 8 batch tiles (PSUM partition dim)
GV = 1024                  # classes per big PSUM unit (2 banks, 4 buffers)
NVG = 13                   # units per batch tile: 12 big + 1 small (256)
SMALL = VSP - 12 * GV      # 256
MARGIN = 0.4
EPS = 1e-7
SX = 32.0                  # fp8 scale for x_norm
SW = 256.0                 # fp8 scale for w_norm
EXP_SCALE = 1.0 / (SX * SW)

# Units are visited CLASS-MAJOR (all 8 b-tiles of class chunk 0, then chunk
# 1, ...) so the weight DMA (~18us for the shard) stays ahead of the PE,
# which consumes each class chunk 8 times (~7us per chunk).
#
# Big units alternate WHOLE-UNIT between the two exp engines: ScalarE
# (exp+accum_out, chain ~1.4us) and VectorE (Schraudolph TENSOR_SCALAR,
# chain ~1.2us). With four 2-bank PSUM buffers the release budget is 3 PE
# periods (~2.6us), so worst-case consumer chains fit with slack and the PE
# never stalls (a stall also re-throttles the HAM clock gate to 1.2 GHz,
# doubling the damage). The Schraudolph units' free-axis sums run on the
# otherwise-idle GpSimd (tensor_scalar accum_out over a 4-slot SBUF ring) --
# both DVE reduces and ScalarE copies are too expensive. Small (256-col)
# units always run on ScalarE.
N_DVE = 48                 # of the 96 big units; ScalarE takes the rest.
                           # 48 = strict A/D alternation: neither engine ever
                           # has two pending chains, so PSUM releases are
                           # worst-case bounded under the 3-period budget.

def _unit_order():
    """[(bt, vg, engine)] in emission order."""
    units = []
    nbig = 0
    for vg in range(11):
        for bt in range(NBT):
            eng = "D" if (nbig * N_DVE) // 96 != ((nbig + 1) * N_DVE) // 96 else "A"
            units.append((bt, vg, eng))
            nbig += 1
    for bt in range(NBT):
        eng = "D" if (nbig * N_DVE) // 96 != ((nbig + 1) * N_DVE) // 96 else "A"
        units.append((bt, 11, eng))
        nbig += 1
        units.append((bt, 12, "A"))
    return units

# Schraudolph constants: bits(bf16(exp(P*EXP_SCALE))) ~= round(SCH_A*P+SCH_B)
SCH_A = 128.0 * math.log2(math.e) * EXP_SCALE
SCH_B = 127.0 * 128.0 - 2.0
# zero-pad classes live in the ScalarE small-group share: exp(0) = 1 exactly
PAD_VAL = 1.0

# graded weight-DMA chunks (class-column bounds): small first chunks so the
# first matmuls start early, big later ones to keep the DGE count low
DMA_BOUNDS = [0, 1024, 2048, 3072, 4096, 6144, 8192, 10240, 12544]
WARMUP_MM = 16             # dummy 256-px matmuls to warm the PE HAM clock
                           # gate and bridge the first weight chunk's DMA

BF16 = mybir.dt.bfloat16
FP8 = mybir.dt.float8e4
U16 = mybir.dt.uint16
F32 = mybir.dt.float32
AF = mybir.ActivationFunctionType
DR = mybir.MatmulPerfMode.DoubleRow
ALU = mybir.AluOpType
AX = mybir.AxisListType

_nc_cache = {}


def _split_multi_waits(nc):
    """This toolchain's walrus accepts at most ONE semaphore wait per
    instruction, but TileContext attaches one wait per producing processor.
    Rewrite any instruction carrying N>1 waits into N-1 same-engine NoOps
    (one wait each) inserted immediately before it; same-engine program order
    keeps the semantics identical."""
    uid = 0
    for f in nc.m.functions:
        for bb in f.blocks:
            insts = bb.instructions
            i = 0
            while i < len(insts):
                inst = insts[i]
                si = inst.sync_info
                if si is not None and len(si.on_wait) > 1:
                    waits = list(si.on_wait)
                    for w in waits[:-1]:
                        uid += 1
                        nop = mybir.InstNoOp(
                            name=f"{inst.name}-wsplit{uid}",
                            engine=inst.engine,
                            sync_info=mybir.SyncInfo(on_wait=[w], on_update=[]),
                            bass_nofuse=True,
                        )
                        insts.insert(i, nop)
                        i += 1
                    inst.sync_info = mybir.SyncInfo(
                        on_wait=[waits[-1]], on_update=list(si.on_update)
                    )
                i += 1


def _build_nc():
    nc = bass.Bass(target_bir_lowering=False)
    xT = nc.declare_dram_parameter("xT", [D, B], FP8, isOutput=False)
    w = nc.declare_dram_parameter("w", [D, VSP], FP8, isOutput=False)
    # ScalarE's per-(b-tile, unit) partial sums, plus the raw Schraudolph
    # exp tiles of the VectorE units -- their free-axis sum happens on the
    # host (no on-device reduce runs at an acceptable rate; DMA is idle)
    acc_out = nc.declare_dram_parameter("acc", [128, NBT * NVG], F32, isOutput=True)
    accd_out = nc.declare_dram_parameter("accd", [128, N_DVE * GV], BF16, isOutput=True)

    xT_r = xT.rearrange("(k p) b -> p k b", p=128)
    w_r = w.rearrange("(k p) v -> p k v", p=128)

    with tile.TileContext(nc) as tc:
        with (
            tc.tile_pool(name="persist", bufs=1) as persist,
            tc.tile_pool(name="pm", bufs=4, space="PSUM") as pm_pool,
        ):
            xt = persist.tile([128, KB, B], FP8, tag="xt")
            # first two b-tiles' stationary x first, so matmuls start early
            nc.sync.dma_start(xt[:, :, :256], xT_r[:, :, :256])
            warm = persist.tile([128, 256], FP8, tag="warm")
            nc.vector.memset(warm[:, :], 0.0625)
            # whole weight shard stays resident in SBUF (fp8: ~6.3 MB)
            wall = persist.tile([128, KB, VSP], FP8, tag="wall")
            v0, v1 = DMA_BOUNDS[0], DMA_BOUNDS[1]
            nc.sync.dma_start(wall[:, :, v0:v1], w_r[:, :, v0:v1])
            nc.sync.dma_start(xt[:, :, 256:], xT_r[:, :, 256:])
            for c in range(1, len(DMA_BOUNDS) - 1):
                v0, v1 = DMA_BOUNDS[c], DMA_BOUNDS[c + 1]
                nc.sync.dma_start(wall[:, :, v0:v1], w_r[:, :, v0:v1])
            scr_a = persist.tile([128, GV], BF16, tag="scr_a")
            scr_d = persist.tile([128, 4, GV], BF16, tag="scr_d")
            sums_a = persist.tile([128, NBT * NVG], F32, tag="sums_a")
            nc.vector.memset(sums_a[:, :], 0.0)
            accd_r = accd_out.rearrange("p (j v) -> p j v", v=GV)

            nd = 0
            for u, (bt, vg, eng) in enumerate(_unit_order()):
                bs = slice(bt * 128, (bt + 1) * 128)
                nv = GV if vg < 12 else SMALL
                c0 = vg * GV
                col = bt * NVG + vg
                psum = pm_pool.tile([128, GV], F32, tag="pm")
                if u == 0:
                    # keep the PE busy during the weight-DMA lead-in so
                    # the HAM clock gate is 8/8 when real matmuls start
                    for _ in range(WARMUP_MM):
                        nc.tensor.matmul(
                            psum[:, :256], warm[:, :128], warm[:, :]
                        )
                for g in range(KB // 2):
                    for c in range(0, nv, 512):
                        cw = min(512, nv - c)
                        nc.tensor.matmul(
                            psum[:, c : c + cw],
                            xt[:, 2 * g : 2 * g + 2, bs],
                            wall[:, 2 * g : 2 * g + 2, c0 + c : c0 + c + cw],
                            start=(g == 0),
                            stop=(g == KB // 2 - 1),
                            perf_mode=DR,
                        )
                if eng == "A":
                    nc.scalar.activation(
                        scr_a[:, :nv],
                        psum[:, :nv],
                        AF.Exp,
                        scale=EXP_SCALE,
                        accum_out=sums_a[:, col : col + 1],
                    )
                else:
                    slot = nd % 4
                    nc.vector.tensor_scalar(
                        scr_d[:, slot, :].bitcast(U16),
                        psum[:, :],
                        SCH_A,
                        SCH_B,
                        op0=ALU.mult,
                        op1=ALU.add,
                    )
                    nc.sync.dma_start(accd_r[:, nd, :], scr_d[:, slot, :])
                    nd += 1

            nc.sync.dma_start(acc_out[:, :], sums_a[:, :])

    _split_multi_waits(nc)
    return nc


def _get_nc():
    if "nc" not in _nc_cache:
        _nc_cache["nc"] = _build_nc()
    return _nc_cache["nc"]


def run_device(in_maps, **kwargs):
    return run_bass_kernel_spmd(_get_nc(), in_maps, list(range(NCORES)), **kwargs)


def make_in_maps(input, weight):
    x = np.asarray(input, dtype=np.float32)
    w = np.asarray(weight, dtype=np.float32)
    x_norm = x / np.maximum(np.linalg.norm(x, axis=1, keepdims=True), 1e-12)
    w_norm = w / np.maximum(np.linalg.norm(w, axis=0, keepdims=True), 1e-12)
    np_dt = ml_dtypes.float8_e4m3
    xT8 = np.ascontiguousarray(x_norm.T * np.float32(SX)).astype(np_dt)
    w8 = (w_norm * np.float32(SW)).astype(np_dt)
    pad = np.zeros((D, VSP - VS), dtype=np_dt)
    return [
        {
            "xT": xT8,
            "w": np.ascontiguousarray(
                np.concatenate([w8[:, i * VS : (i + 1) * VS], pad], axis=1)
            ),
        }
        for i in range(NCORES)
    ]


def finalize(results, input, weight, labels):
    """Host epilogue: reduce the per-(b-tile, group) partial sums, remove the
    zero-pad columns' exact contributions, and apply the exact label-margin
    correction (O(B*D) work)."""
    x = np.asarray(input, dtype=np.float64)
    w = np.asarray(weight, dtype=np.float32)
    lab = np.asarray(labels).astype(np.int64)

    d_bts = [bt for bt, vg, eng in _unit_order() if eng == "D"]
    S = np.zeros(B, dtype=np.float64)
    for i in range(NCORES):
        part = (
            results[i]["acc"].astype(np.float64).reshape(128, NBT, NVG).sum(axis=2)
        )
        dsum = (
            results[i]["accd"].astype(np.float64).reshape(128, N_DVE, GV).sum(axis=2)
        )
        for j, bt in enumerate(d_bts):
            part[:, bt] += dsum[:, j]
        S += part.T.reshape(B)
    # zero-pad classes sit in the ScalarE small-group share: exp(0) = 1
    S -= NCORES * (VSP - VS) * PAD_VAL

    x_norm = x / np.maximum(np.linalg.norm(x, axis=1, keepdims=True), 1e-12)
    wl = w[:, lab].astype(np.float64)                    # [D, B]
    wln = np.maximum(np.sqrt((wl * wl).sum(axis=0)), 1e-12)
    c = (x_norm.T * wl).sum(axis=0) / wln                # label cosines
    c = np.clip(c, -1.0 + EPS, 1.0 - EPS)
    c_adj = np.cos(np.arccos(c) + MARGIN)
    S_adj = S - np.exp(c) + np.exp(c_adj)
    logz = np.log(S_adj)
    loss = np.mean(logz - c_adj)
    return np.asarray(loss, dtype=np.float32)


def kernel(input, weight, labels):
    in_maps = make_in_maps(input, weight)
    res = run_device(in_maps)
    return finalize(res.results, input, weight, labels)


# revision 48
# speedup vs baseline: 1.1568x; 1.1568x over previous
"""ArcMarginProduct + cross-entropy loss, vocab-parallel over 8 NeuronCores.

Math: the reference computes
    cos[b,v] = <x_b/|x_b|, w_v/|w_v|>,  clip to [-1+eps, 1-eps]
    logits   = cos(arccos(cos) + M*onehot(labels))
    loss     = mean(logsumexp(logits, axis=1) - logits[b, label_b])
For v != label_b, cos(arccos(c)) == c, so the only place arccos/cos matter is
the single label column per row -- handled exactly on the host (O(B*D) work).
The device computes, per vocabulary shard, S_partial[b] = sum_v exp(cos[b,v])
(raw, no margin). |cos|<=1 always, so no max-shift is needed for stability.
Host then corrects the label term: S_adj = S - exp(c_label) + exp(c_adj),
loss = mean(log(S_adj) - c_adj).

Sharding: weight columns split V=100000 -> 8 x 12500, padded with zero
columns to 12544 per core (pad contributions are constant and subtracted
exactly on the host).

Device kernel (per core): both operands are L2-normalized ON THE HOST and
shipped as fp8, so the PSUM matmul result is exactly SX*SW*cos and the exp
scale is one scalar constant. Layout: batch rows on PSUM partitions (8
b-tiles of 128), classes on the free axis, so the per-row sum over classes
is a free-axis reduction the ScalarE activation produces for free via
accum_out. Per [128, 2048] class group (4 PSUM banks, double-buffered):
DoubleRow fp8 matmuls (256-deep contraction, x stationary) accumulate over
D; then the group is consumed column-split by TWO engines in parallel --
ScalarE runs Exp+accum_out on the first ACT_COLS columns while VectorE runs
a Schraudolph bit-trick exp (bits(bf16(e^z)) ~= round(A*P + B), ~1.3% rms,
~0.1% mean error) on the rest, with one batched free-axis reduce per b-tile.
No SBUF-side accumulation pass exists at all; the host sums 7 columns per
b-tile. PE warm-up matmuls during the weight-DMA lead-in keep the HAM clock
gate at 8/8 when the real matmuls start.
"""

import math
import sys

if "/opt/trn_rl_repo" not in sys.path:
    sys.path.insert(0, "/opt/trn_rl_repo")

import numpy as np
import ml_dtypes

import concourse.bass as bass
import concourse.mybir as mybir
import concourse.tile as tile
from concourse.bass_utils import run_bass_kernel_spmd

B, D, V = 1024, 512, 100000
NCORES = 8
VS = V // NCORES           # 12500 true classes per core
VSP = 12544                # padded classes per core
KB = D // 128              # 4 contraction blocks
NBT = B // 128             # 8 batch tiles (PSUM partition dim)
GV = 1024                  # classes per big PSUM unit (2 banks, 4 buffers)
NVG = 13                   # units per batch tile: 12 big + 1 small (256)
SMALL = VSP - 12 * GV      # 256
MARGIN = 0.4
EPS = 1e-7
SX = 32.0                  # fp8 scale for x_norm
SW = 256.0                 # fp8 scale for w_norm
EXP_SCALE = 1.0 / (SX * SW)

# Units are visited CLASS-MAJOR (all 8 b-tiles of class chunk 0, then chunk
# 1, ...) so the weight DMA (~18us for the shard) stays ahead of the PE,
# which consumes each class chunk 8 times (~7us per chunk).
#
# Big units alternate WHOLE-UNIT between the two exp engines: ScalarE
# (exp+accum_out, chain ~1.4us) and VectorE (Schraudolph TENSOR_SCALAR,
# chain ~1.2us). With four 2-bank PSUM buffers the release budget is 3 PE
# periods (~2.6us), so worst-case consumer chains fit with slack and the PE
# never stalls (a stall also re-throttles the HAM clock gate to 1.2 GHz,
# doubling the damage). The Schraudolph units' free-axis sums run on the
# otherwise-idle GpSimd (tensor_scalar accum_out over a 4-slot SBUF ring) --
# both DVE reduces and ScalarE copies are too expensive. Small (256-col)
# units always run on ScalarE.
N_DVE = 48                 # of the 96 big units; ScalarE takes the rest.
                           # 48 = strict A/D alternation: neither engine ever
                           # has two pending chains, so PSUM releases are
                           # worst-case bounded under the 3-period budget.

def _unit_order():
    """[(bt, vg, engine)] in emission order."""
    units = []
    nbig = 0
    for vg in range(11):
        for bt in range(NBT):
            eng = "D" if (nbig * N_DVE) // 96 != ((nbig + 1) * N_DVE) // 96 else "A"
            units.append((bt, vg, eng))
            nbig += 1
    for bt in range(NBT):
        eng = "D" if (nbig * N_DVE) // 96 != ((nbig + 1) * N_DVE) // 96 else "A"
        units.append((bt, 11, eng))
        nbig += 1
        units.append((bt, 12, "A"))
    return units

# Schraudolph constants, fp8e4m3 flavor: the uint8 bit pattern of
# fp8(exp(P*EXP_SCALE)) ~= round(SCH_A*P + SCH_B). ~3% rms / ~0.1% mean
# per-element error; fp8 keeps the DMA-out of the VectorE units' exp tiles
# at 6 MB/core -- bf16 tiles measurably tipped the chip into the P0 power
# state (every engine clock -17%).
SCH_A = 8.0 * math.log2(math.e) * EXP_SCALE
SCH_B = 7.0 * 8.0 - 0.11
# zero-pad classes live in the ScalarE small-group share: exp(0) = 1 exactly
PAD_VAL = 1.0

# graded weight-DMA chunks (class-column bounds): small first chunks so the
# first matmuls start early, big later ones to keep the DGE count low
DMA_BOUNDS = [0, 1024, 2048, 3072, 4096, 6144, 8192, 10240, 12544]
WARMUP_MM = 16             # dummy 256-px matmuls to warm the PE HAM clock
                           # gate and bridge the first weight chunk's DMA

BF16 = mybir.dt.bfloat16
FP8 = mybir.dt.float8e4
U16 = mybir.dt.uint16
U8 = mybir.dt.uint8
F32 = mybir.dt.float32
AF = mybir.ActivationFunctionType
DR = mybir.MatmulPerfMode.DoubleRow
ALU = mybir.AluOpType
AX = mybir.AxisListType

_nc_cache = {}


def _split_multi_waits(nc):
    """This toolchain's walrus accepts at most ONE semaphore wait per
    instruction, but TileContext attaches one wait per producing processor.
    Rewrite any instruction carrying N>1 waits into N-1 same-engine NoOps
    (one wait each) inserted immediately before it; same-engine program order
    keeps the semantics identical."""
    uid = 0
    for f in nc.m.functions:
        for bb in f.blocks:
            insts = bb.instructions
            i = 0
            while i < len(insts):
                inst = insts[i]
                si = inst.sync_info
                if si is not None and len(si.on_wait) > 1:
                    waits = list(si.on_wait)
                    for w in waits[:-1]:
                        uid += 1
                        nop = mybir.InstNoOp(
                            name=f"{inst.name}-wsplit{uid}",
                            engine=inst.engine,
                            sync_info=mybir.SyncInfo(on_wait=[w], on_update=[]),
                            bass_nofuse=True,
                        )
                        insts.insert(i, nop)
                        i += 1
                    inst.sync_info = mybir.SyncInfo(
                        on_wait=[waits[-1]], on_update=list(si.on_update)
                    )
                i += 1


def _build_nc():
    nc = bass.Bass(target_bir_lowering=False)
    xT = nc.declare_dram_parameter("xT", [D, B], FP8, isOutput=False)
    w = nc.declare_dram_parameter("w", [D, VSP], FP8, isOutput=False)
    # ScalarE's per-(b-tile, unit) partial sums, plus the raw Schraudolph
    # exp tiles of the VectorE units -- their free-axis sum happens on the
    # host (no on-device reduce runs at an acceptable rate; DMA is idle)
    acc_out = nc.declare_dram_parameter("acc", [128, NBT * NVG], F32, isOutput=True)
    accd_out = nc.declare_dram_parameter("accd", [128, N_DVE * GV], U8, isOutput=True)

    xT_r = xT.rearrange("(k p) b -> p k b", p=128)
    w_r = w.rearrange("(k p) v -> p k v", p=128)

    with tile.TileContext(nc) as tc:
        with (
            tc.tile_pool(name="persist", bufs=1) as persist,
            tc.tile_pool(name="pm", bufs=4, space="PSUM") as pm_pool,
        ):
            xt = persist.tile([128, KB, B], FP8, tag="xt")
            # first two b-tiles' stationary x first, so matmuls start early
            nc.sync.dma_start(xt[:, :, :256], xT_r[:, :, :256])
            warm = persist.tile([128, 256], FP8, tag="warm")
            nc.vector.memset(warm[:, :], 0.0625)
            # whole weight shard stays resident in SBUF (fp8: ~6.3 MB)
            wall = persist.tile([128, KB, VSP], FP8, tag="wall")
            v0, v1 = DMA_BOUNDS[0], DMA_BOUNDS[1]
            nc.sync.dma_start(wall[:, :, v0:v1], w_r[:, :, v0:v1])
            nc.sync.dma_start(xt[:, :, 256:], xT_r[:, :, 256:])
            for c in range(1, len(DMA_BOUNDS) - 1):
                v0, v1 = DMA_BOUNDS[c], DMA_BOUNDS[c + 1]
                nc.sync.dma_start(wall[:, :, v0:v1], w_r[:, :, v0:v1])
            scr_a = persist.tile([128, GV], BF16, tag="scr_a")
            scr_d = persist.tile([128, 4, GV], U8, tag="scr_d")
            sums_a = persist.tile([128, NBT * NVG], F32, tag="sums_a")
            nc.vector.memset(sums_a[:, :], 0.0)
            accd_r = accd_out.rearrange("p (j v) -> p j v", v=GV)

            nd = 0
            for u, (bt, vg, eng) in enumerate(_unit_order()):
                bs = slice(bt * 128, (bt + 1) * 128)
                nv = GV if vg < 12 else SMALL
                c0 = vg * GV
                col = bt * NVG + vg
                psum = pm_pool.tile([128, GV], F32, tag="pm")
                if u == 0:
                    # keep the PE busy during the weight-DMA lead-in so
                    # the HAM clock gate is 8/8 when real matmuls start
                    for _ in range(WARMUP_MM):
                        nc.tensor.matmul(
                            psum[:, :256], warm[:, :128], warm[:, :]
                        )
                for g in range(KB // 2):
                    for c in range(0, nv, 512):
                        cw = min(512, nv - c)
                        nc.tensor.matmul(
                            psum[:, c : c + cw],
                            xt[:, 2 * g : 2 * g + 2, bs],
                            wall[:, 2 * g : 2 * g + 2, c0 + c : c0 + c + cw],
                            start=(g == 0),
                            stop=(g == KB // 2 - 1),
                            perf_mode=DR,
                        )
                if eng == "A":
                    nc.scalar.activation(
                        scr_a[:, :nv],
                        psum[:, :nv],
                        AF.Exp,
                        scale=EXP_SCALE,
                        accum_out=sums_a[:, col : col + 1],
                    )
                else:
                    slot = nd % 4
                    nc.vector.tensor_scalar(
                        scr_d[:, slot, :],
                        psum[:, :],
                        SCH_A,
                        SCH_B,
                        op0=ALU.mult,
                        op1=ALU.add,
                    )
                    nc.sync.dma_start(accd_r[:, nd, :], scr_d[:, slot, :])
                    nd += 1

            nc.sync.dma_start(acc_out[:, :], sums_a[:, :])

    _split_multi_waits(nc)
    return nc


def _get_nc():
    if "nc" not in _nc_cache:
        _nc_cache["nc"] = _build_nc()
    return _nc_cache["nc"]


def run_device(in_maps, **kwargs):
    return run_bass_kernel_spmd(_get_nc(), in_maps, list(range(NCORES)), **kwargs)


def make_in_maps(input, weight):
    x = np.asarray(input, dtype=np.float32)
    w = np.asarray(weight, dtype=np.float32)
    x_norm = x / np.maximum(np.linalg.norm(x, axis=1, keepdims=True), 1e-12)
    w_norm = w / np.maximum(np.linalg.norm(w, axis=0, keepdims=True), 1e-12)
    np_dt = ml_dtypes.float8_e4m3
    xT8 = np.ascontiguousarray(x_norm.T * np.float32(SX)).astype(np_dt)
    w8 = (w_norm * np.float32(SW)).astype(np_dt)
    pad = np.zeros((D, VSP - VS), dtype=np_dt)
    return [
        {
            "xT": xT8,
            "w": np.ascontiguousarray(
                np.concatenate([w8[:, i * VS : (i + 1) * VS], pad], axis=1)
            ),
        }
        for i in range(NCORES)
    ]


def finalize(results, input, weight, labels):
    """Host epilogue: reduce the per-(b-tile, group) partial sums, remove the
    zero-pad columns' exact contributions, and apply the exact label-margin
    correction (O(B*D) work)."""
    x = np.asarray(input, dtype=np.float64)
    w = np.asarray(weight, dtype=np.float32)
    lab = np.asarray(labels).astype(np.int64)

    d_bts = [bt for bt, vg, eng in _unit_order() if eng == "D"]
    S = np.zeros(B, dtype=np.float64)
    for i in range(NCORES):
        part = (
            results[i]["acc"].astype(np.float64).reshape(128, NBT, NVG).sum(axis=2)
        )
        dsum = (
            results[i]["accd"]
            .view(ml_dtypes.float8_e4m3)
            .astype(np.float64)
            .reshape(128, N_DVE, GV)
            .sum(axis=2)
        )
        for j, bt in enumerate(d_bts):
            part[:, bt] += dsum[:, j]
        S += part.T.reshape(B)
    # zero-pad classes sit in the ScalarE small-group share: exp(0) = 1
    S -= NCORES * (VSP - VS) * PAD_VAL

    x_norm = x / np.maximum(np.linalg.norm(x, axis=1, keepdims=True), 1e-12)
    wl = w[:, lab].astype(np.float64)                    # [D, B]
    wln = np.maximum(np.sqrt((wl * wl).sum(axis=0)), 1e-12)
    c = (x_norm.T * wl).sum(axis=0) / wln                # label cosines
    c = np.clip(c, -1.0 + EPS, 1.0 - EPS)
    c_adj = np.cos(np.arccos(c) + MARGIN)
    S_adj = S - np.exp(c) + np.exp(c_adj)
    logz = np.log(S_adj)
    loss = np.mean(logz - c_adj)
    return np.asarray(loss, dtype=np.float32)


def kernel(input, weight, labels):
    in_maps = make_in_maps(input, weight)
    res = run_device(in_maps)
    return finalize(res.results, input, weight, labels)


# revision 51
# speedup vs baseline: 1.1874x; 1.0265x over previous
"""ArcMarginProduct + cross-entropy loss, vocab-parallel over 8 NeuronCores.

Math: the reference computes
    cos[b,v] = <x_b/|x_b|, w_v/|w_v|>,  clip to [-1+eps, 1-eps]
    logits   = cos(arccos(cos) + M*onehot(labels))
    loss     = mean(logsumexp(logits, axis=1) - logits[b, label_b])
For v != label_b, cos(arccos(c)) == c, so the only place arccos/cos matter is
the single label column per row -- handled exactly on the host (O(B*D) work).
The device computes, per vocabulary shard, S_partial[b] = sum_v exp(cos[b,v])
(raw, no margin). |cos|<=1 always, so no max-shift is needed for stability.
Host then corrects the label term: S_adj = S - exp(c_label) + exp(c_adj),
loss = mean(log(S_adj) - c_adj).

Sharding: weight columns split V=100000 -> 8 x 12500, padded with zero
columns to 12544 per core (pad contributions are constant and subtracted
exactly on the host).

Device kernel (per core): both operands are L2-normalized ON THE HOST and
shipped as fp8, so the PSUM matmul result is exactly SX*SW*cos and the exp
scale is one scalar constant. Layout: batch rows on PSUM partitions (8
b-tiles of 128), classes on the free axis, so the per-row sum over classes
is a free-axis reduction the ScalarE activation produces for free via
accum_out. Per [128, 2048] class group (4 PSUM banks, double-buffered):
DoubleRow fp8 matmuls (256-deep contraction, x stationary) accumulate over
D; then the group is consumed column-split by TWO engines in parallel --
ScalarE runs Exp+accum_out on the first ACT_COLS columns while VectorE runs
a Schraudolph bit-trick exp (bits(bf16(e^z)) ~= round(A*P + B), ~1.3% rms,
~0.1% mean error) on the rest, with one batched free-axis reduce per b-tile.
No SBUF-side accumulation pass exists at all; the host sums 7 columns per
b-tile. PE warm-up matmuls during the weight-DMA lead-in keep the HAM clock
gate at 8/8 when the real matmuls start.
"""

import math
import sys

if "/opt/trn_rl_repo" not in sys.path:
    sys.path.insert(0, "/opt/trn_rl_repo")

import numpy as np
import ml_dtypes

import concourse.bass as bass
import concourse.mybir as mybir
import concourse.tile as tile
from concourse.bass_utils import run_bass_kernel_spmd

B, D, V = 1024, 512, 100000
NCORES = 8
VS = V // NCORES           # 12500 true classes per core
VSP = 12544                # padded classes per core
KB = D // 128              # 4 contraction blocks
NBT = B // 128             # 8 batch tiles (PSUM partition dim)
GV = 1024                  # classes per big PSUM unit (2 banks, 4 buffers)
NVG = 13                   # units per batch tile: 12 big + 1 small (256)
SMALL = VSP - 12 * GV      # 256
MARGIN = 0.4
EPS = 1e-7
SX = 32.0                  # fp8 scale for x_norm
SW = 256.0                 # fp8 scale for w_norm
EXP_SCALE = 1.0 / (SX * SW)

# Units are visited CLASS-MAJOR (all 8 b-tiles of class chunk 0, then chunk
# 1, ...) so the weight DMA (~18us for the shard) stays ahead of the PE,
# which consumes each class chunk 8 times (~7us per chunk).
#
# Big units alternate WHOLE-UNIT between the two exp engines: ScalarE
# (exp+accum_out, chain ~1.4us) and VectorE (Schraudolph TENSOR_SCALAR,
# chain ~1.2us). With four 2-bank PSUM buffers the release budget is 3 PE
# periods (~2.6us), so worst-case consumer chains fit with slack and the PE
# never stalls (a stall also re-throttles the HAM clock gate to 1.2 GHz,
# doubling the damage). The Schraudolph units' free-axis sums run on the
# otherwise-idle GpSimd (tensor_scalar accum_out over a 4-slot SBUF ring) --
# both DVE reduces and ScalarE copies are too expensive. Small (256-col)
# units always run on ScalarE.
N_DVE = 50                 # of the 96 big units (36 alternating + 14 tail);
                           # near-strict A/D alternation keeps both engines'
                           # pending chains short, so PSUM releases stay
                           # worst-case bounded under the 3-period budget.

def _unit_order():
    """[(bt, vg, engine)] in emission order.

    Sweeps vg 0..8 alternate A/D strictly. The 8 small units (vg 12, always
    ScalarE, ~0.76us chains) are spread through the last three sweeps (one
    after every third big unit), whose big-unit mix shifts to 14/24 DVE so
    ScalarE absorbs the smalls without backlogging."""
    units = []
    for vg in range(9):
        for bt in range(NBT):
            units.append((bt, vg, "D" if (vg * NBT + bt) % 2 else "A"))
    tail_d = [True, False, True, True, False, True, False, True, True, False, True, False]
    smalls = iter(range(NBT))
    nbig = 0
    for vg in range(9, 12):
        for bt in range(NBT):
            units.append((bt, vg, "D" if tail_d[nbig % 12] else "A"))
            nbig += 1
            if nbig % 3 == 0:
                units.append((next(smalls), 12, "A"))
    return units

# Schraudolph constants, fp8e4m3 flavor: the uint8 bit pattern of
# fp8(exp(P*EXP_SCALE)) ~= round(SCH_A*P + SCH_B). ~3% rms / ~0.1% mean
# per-element error; fp8 keeps the DMA-out of the VectorE units' exp tiles
# at 6 MB/core -- bf16 tiles measurably tipped the chip into the P0 power
# state (every engine clock -17%).
SCH_A = 8.0 * math.log2(math.e) * EXP_SCALE
SCH_B = 7.0 * 8.0 - 0.11
# zero-pad classes live in the ScalarE small-group share: exp(0) = 1 exactly
PAD_VAL = 1.0

# graded weight-DMA chunks (class-column bounds): small first chunks so the
# first matmuls start early, big later ones to keep the DGE count low
DMA_BOUNDS = [0, 1024, 2048, 3072, 4096, 6144, 8192, 10240, 12544]
WARMUP_MM = 20             # dummy 256-px matmuls to warm the PE HAM clock
                           # gate and bridge the first weight chunk's DMA

BF16 = mybir.dt.bfloat16
FP8 = mybir.dt.float8e4
U16 = mybir.dt.uint16
U8 = mybir.dt.uint8
F32 = mybir.dt.float32
AF = mybir.ActivationFunctionType
DR = mybir.MatmulPerfMode.DoubleRow
ALU = mybir.AluOpType
AX = mybir.AxisListType

_nc_cache = {}


def _split_multi_waits(nc):
    """This toolchain's walrus accepts at most ONE semaphore wait per
    instruction, but TileContext attaches one wait per producing processor.
    Rewrite any instruction carrying N>1 waits into N-1 same-engine NoOps
    (one wait each) inserted immediately before it; same-engine program order
    keeps the semantics identical."""
    uid = 0
    for f in nc.m.functions:
        for bb in f.blocks:
            insts = bb.instructions
            i = 0
            while i < len(insts):
                inst = insts[i]
                si = inst.sync_info
                if si is not None and len(si.on_wait) > 1:
                    waits = list(si.on_wait)
                    for w in waits[:-1]:
                        uid += 1
                        nop = mybir.InstNoOp(
                            name=f"{inst.name}-wsplit{uid}",
                            engine=inst.engine,
                            sync_info=mybir.SyncInfo(on_wait=[w], on_update=[]),
                            bass_nofuse=True,
                        )
                        insts.insert(i, nop)
                        i += 1
                    inst.sync_info = mybir.SyncInfo(
                        on_wait=[waits[-1]], on_update=list(si.on_update)
                    )
                i += 1


def _build_nc():
    nc = bass.Bass(target_bir_lowering=False)
    xT = nc.declare_dram_parameter("xT", [D, B], FP8, isOutput=False)
    w = nc.declare_dram_parameter("w", [D, VSP], FP8, isOutput=False)
    # ScalarE's per-(b-tile, unit) partial sums, plus the raw Schraudolph
    # exp tiles of the VectorE units -- their free-axis sum happens on the
    # host (no on-device reduce runs at an acceptable rate; DMA is idle)
    acc_out = nc.declare_dram_parameter("acc", [128, NBT * NVG], F32, isOutput=True)
    accd_out = nc.declare_dram_parameter("accd", [128, N_DVE * GV], U8, isOutput=True)

    xT_r = xT.rearrange("(k p) b -> p k b", p=128)
    w_r = w.rearrange("(k p) v -> p k v", p=128)

    with tile.TileContext(nc) as tc:
        with (
            tc.tile_pool(name="persist", bufs=1) as persist,
            tc.tile_pool(name="pm", bufs=4, space="PSUM") as pm_pool,
        ):
            xt = persist.tile([128, KB, B], FP8, tag="xt")
            # first two b-tiles' stationary x first, so matmuls start early
            nc.sync.dma_start(xt[:, :, :256], xT_r[:, :, :256])
            warm = persist.tile([128, 256], FP8, tag="warm")
            nc.vector.memset(warm[:, :], 0.0625)
            # whole weight shard stays resident in SBUF (fp8: ~6.3 MB)
            wall = persist.tile([128, KB, VSP], FP8, tag="wall")
            v0, v1 = DMA_BOUNDS[0], DMA_BOUNDS[1]
            nc.sync.dma_start(wall[:, :, v0:v1], w_r[:, :, v0:v1])
            nc.sync.dma_start(xt[:, :, 256:], xT_r[:, :, 256:])
            for c in range(1, len(DMA_BOUNDS) - 1):
                v0, v1 = DMA_BOUNDS[c], DMA_BOUNDS[c + 1]
                nc.sync.dma_start(wall[:, :, v0:v1], w_r[:, :, v0:v1])
            scr_a = persist.tile([128, GV], BF16, tag="scr_a")
            scr_d = persist.tile([128, 4, GV], U8, tag="scr_d")
            sums_a = persist.tile([128, NBT * NVG], F32, tag="sums_a")
            nc.vector.memset(sums_a[:, :], 0.0)
            accd_r = accd_out.rearrange("p (j v) -> p j v", v=GV)

            nd = 0
            for u, (bt, vg, eng) in enumerate(_unit_order()):
                bs = slice(bt * 128, (bt + 1) * 128)
                nv = GV if vg < 12 else SMALL
                c0 = vg * GV
                col = bt * NVG + vg
                psum = pm_pool.tile([128, GV], F32, tag="pm")
                if u == 0:
                    # keep the PE busy during the weight-DMA lead-in so
                    # the HAM clock gate is 8/8 when real matmuls start
                    for _ in range(WARMUP_MM):
                        nc.tensor.matmul(
                            psum[:, :256], warm[:, :128], warm[:, :]
                        )
                for g in range(KB // 2):
                    for c in range(0, nv, 512):
                        cw = min(512, nv - c)
                        nc.tensor.matmul(
                            psum[:, c : c + cw],
                            xt[:, 2 * g : 2 * g + 2, bs],
                            wall[:, 2 * g : 2 * g + 2, c0 + c : c0 + c + cw],
                            start=(g == 0),
                            stop=(g == KB // 2 - 1),
                            perf_mode=DR,
                        )
                if eng == "A":
                    nc.scalar.activation(
                        scr_a[:, :nv],
                        psum[:, :nv],
                        AF.Exp,
                        scale=EXP_SCALE,
                        accum_out=sums_a[:, col : col + 1],
                    )
                else:
                    slot = nd % 4
                    nc.vector.tensor_scalar(
                        scr_d[:, slot, :],
                        psum[:, :],
                        SCH_A,
                        SCH_B,
                        op0=ALU.mult,
                        op1=ALU.add,
                    )
                    nc.sync.dma_start(accd_r[:, nd, :], scr_d[:, slot, :])
                    nd += 1

            nc.sync.dma_start(acc_out[:, :], sums_a[:, :])

    _split_multi_waits(nc)
    return nc


def _get_nc():
    if "nc" not in _nc_cache:
        _nc_cache["nc"] = _build_nc()
    return _nc_cache["nc"]


def run_device(in_maps, **kwargs):
    return run_bass_kernel_spmd(_get_nc(), in_maps, list(range(NCORES)), **kwargs)


def make_in_maps(input, weight):
    x = np.asarray(input, dtype=np.float32)
    w = np.asarray(weight, dtype=np.float32)
    x_norm = x / np.maximum(np.linalg.norm(x, axis=1, keepdims=True), 1e-12)
    w_norm = w / np.maximum(np.linalg.norm(w, axis=0, keepdims=True), 1e-12)
    np_dt = ml_dtypes.float8_e4m3
    xT8 = np.ascontiguousarray(x_norm.T * np.float32(SX)).astype(np_dt)
    w8 = (w_norm * np.float32(SW)).astype(np_dt)
    pad = np.zeros((D, VSP - VS), dtype=np_dt)
    return [
        {
            "xT": xT8,
            "w": np.ascontiguousarray(
                np.concatenate([w8[:, i * VS : (i + 1) * VS], pad], axis=1)
            ),
        }
        for i in range(NCORES)
    ]


def finalize(results, input, weight, labels):
    """Host epilogue: reduce the per-(b-tile, group) partial sums, remove the
    zero-pad columns' exact contributions, and apply the exact label-margin
    correction (O(B*D) work)."""
    x = np.asarray(input, dtype=np.float64)
    w = np.asarray(weight, dtype=np.float32)
    lab = np.asarray(labels).astype(np.int64)

    d_bts = [bt for bt, vg, eng in _unit_order() if eng == "D"]
    S = np.zeros(B, dtype=np.float64)
    for i in range(NCORES):
        part = (
            results[i]["acc"].astype(np.float64).reshape(128, NBT, NVG).sum(axis=2)
        )
        dsum = (
            results[i]["accd"]
            .view(ml_dtypes.float8_e4m3)
            .astype(np.float64)
            .reshape(128, N_DVE, GV)
            .sum(axis=2)
        )
        for j, bt in enumerate(d_bts):
            part[:, bt] += dsum[:, j]
        S += part.T.reshape(B)
    # zero-pad classes sit in the ScalarE small-group share: exp(0) = 1
    S -= NCORES * (VSP - VS) * PAD_VAL

    x_norm = x / np.maximum(np.linalg.norm(x, axis=1, keepdims=True), 1e-12)
    wl = w[:, lab].astype(np.float64)                    # [D, B]
    wln = np.maximum(np.sqrt((wl * wl).sum(axis=0)), 1e-12)
    c = (x_norm.T * wl).sum(axis=0) / wln                # label cosines
    c = np.clip(c, -1.0 + EPS, 1.0 - EPS)
    c_adj = np.cos(np.arccos(c) + MARGIN)
    S_adj = S - np.exp(c) + np.exp(c_adj)
    logz = np.log(S_adj)
    loss = np.mean(logz - c_adj)
    return np.asarray(loss, dtype=np.float32)


def kernel(input, weight, labels):
    in_maps = make_in_maps(input, weight)
    res = run_device(in_maps)
    return finalize(res.results, input, weight, labels)
